# revision 4
# baseline (speedup 1.0000x reference)
"""Trainium2 Bass kernel for nn_O3TensorProductWeighted.

Computes, for each sample e:
    h  = relu(relu(weight @ W0 + b0) @ W1 + b1)           # [64]
    w  = h @ W2 + b2                                      # [36864] (never materialized)
    out0 = PW0*(einsum(Wa,s1)*s2 + I3*einsum(Wd,dot12))
    out1 = PW1*I3*(einsum(Wb,s1) x v2 + einsum(Wc,v1)*s2)
    out  = concat(out0, out1)/SQRT_K ; out[:128] += bias

Key reassociation: einsum('euw,eu->ew', Wa, s1) with Wa = h@W2a + b2a is
computed as (h (x) s1) @ W2a_flat + s1 @ b2a — a dense matmul over the joint
(k,u) contraction index, with the per-sample Khatri-Rao product h (x) s1
built on-chip (PE broadcast trick + DVE multiply). Everything runs in
transposed layout (features on partitions, samples on the free dim), 512
samples per core, pure data parallel over 8 cores.
"""

import sys

sys.path.insert(0, "/opt/trn_rl_repo")

from contextlib import ExitStack

import ml_dtypes
import numpy as np

import concourse.bass as bass
import concourse.bacc as bacc
import concourse.tile as tile
from concourse import mybir
from concourse.bass_utils import run_bass_kernel_spmd

BF16 = mybir.dt.bfloat16
F32 = mybir.dt.float32
BF16_NP = ml_dtypes.bfloat16

N_CORES = 8
N = 4096
E = N // N_CORES  # 512 samples per core

MUL0, MUL1 = 128, 64
N1 = MUL0 * MUL0          # 16384
N2 = MUL0 * MUL1          # 8192
N3 = MUL1 * MUL1          # 4096
N4 = MUL1 * MUL0          # 8192
FAN = MUL0 + MUL1         # 192
I3 = float(1.0 / np.sqrt(3.0))
# PW0/SQRT_K == 1.0 and PW1*I3/SQRT_K == 1.0 exactly, so no output scales
# are needed beyond I3 on the D path.

KAB = 64   # number of 128-row (k,u) chunks for paths a/b (k-major, u full 128)
GCD = 32   # number of 128-row chunks for paths c/d (2 k values x 64 u)


def _build_nc():
    nc = bacc.Bacc(None)

    # ---- per-core (sample-sliced) inputs, transposed: [feature, E] ----
    s1t_d = nc.declare_dram_parameter("s1t", [128, E], BF16, isOutput=False)
    vs_d = [
        nc.declare_dram_parameter(f"vs{i}", [128, E], BF16, isOutput=False)
        for i in range(3)
    ]  # [v1s2_iT; v1s2_iT] stacked pair
    d2_d = nc.declare_dram_parameter("d2", [128, E], BF16, isOutput=False)
    wT_d = nc.declare_dram_parameter("wT", [16, E], BF16, isOutput=False)
    s2b_d = nc.declare_dram_parameter("s2b", [128, E], F32, isOutput=False)
    v2b_d = [
        nc.declare_dram_parameter(f"v2b{i}", [64, E], F32, isOutput=False)
        for i in range(3)
    ]

    # ---- replicated parameters ----
    w0_d = nc.declare_dram_parameter("w0", [16, 64], BF16, isOutput=False)
    b0c_d = nc.declare_dram_parameter("b0c", [64, 1], F32, isOutput=False)
    w1ab_d = nc.declare_dram_parameter("w1ab", [64, KAB * 128], BF16, isOutput=False)
    w1cd_d = nc.declare_dram_parameter("w1cd", [64, GCD * 128], BF16, isOutput=False)
    b1ab_d = nc.declare_dram_parameter("b1ab", [128, KAB], F32, isOutput=False)
    b1cd_d = nc.declare_dram_parameter("b1cd", [128, GCD], F32, isOutput=False)
    wa_d = nc.declare_dram_parameter("wa", [128, KAB * 128], BF16, isOutput=False)
    wb_d = nc.declare_dram_parameter("wb", [128, KAB * 64], BF16, isOutput=False)
    wc_d = nc.declare_dram_parameter("wc", [128, GCD * 64], BF16, isOutput=False)
    wd_d = nc.declare_dram_parameter("wd", [128, GCD * 128], BF16, isOutput=False)
    ba_d = nc.declare_dram_parameter("ba", [128, 128], BF16, isOutput=False)
    bb_d = nc.declare_dram_parameter("bb", [128, 64], BF16, isOutput=False)
    bc_d = nc.declare_dram_parameter("bc", [64, 64], BF16, isOutput=False)
    bd_d = nc.declare_dram_parameter("bd", [64, 128], BF16, isOutput=False)
    bcol_d = nc.declare_dram_parameter("bcol", [128, 1], F32, isOutput=False)
    ident_d = nc.declare_dram_parameter("ident", [128, 128], F32, isOutput=False)

    outp_d = nc.declare_dram_parameter("outp", [E, 320], F32, isOutput=True)

    with tile.TileContext(nc) as tc, ExitStack() as ctx:
        const = ctx.enter_context(tc.tile_pool(name="const", bufs=1))
        work = ctx.enter_context(tc.tile_pool(name="work", bufs=1))
        bct_pool = ctx.enter_context(tc.tile_pool(name="bct", bufs=3))
        pt_pool = ctx.enter_context(tc.tile_pool(name="pt", bufs=4))
        out_pool = ctx.enter_context(tc.tile_pool(name="outs", bufs=2))
        ps_acc = ctx.enter_context(
            tc.tile_pool(name="ps_acc", bufs=1, space="PSUM")
        )
        ps_rot = ctx.enter_context(
            tc.tile_pool(name="ps_rot", bufs=2, space="PSUM")
        )

        def load(pool, dparam, dtype=None):
            t = pool.tile(dparam.shape, dtype or dparam.dtype,
                          name=f"t_{dparam.name}")
            nc.sync.dma_start(t[:], dparam[:])
            return t

        # small inputs first so the MLP can start immediately
        wT_t = load(const, wT_d)
        w0_t = load(const, w0_d)
        b0c_t = load(const, b0c_d)
        s1t_t = load(const, s1t_d)
        vs_t = [load(const, d) for d in vs_d]
        d2_t = load(const, d2_d)
        b1ab_t = load(const, b1ab_d)
        b1cd_t = load(const, b1cd_d)
        ba_t = load(const, ba_d)
        bb_t = load(const, bb_d)
        bc_t = load(const, bc_d)
        bd_t = load(const, bd_d)
        s2b_t = load(const, s2b_d)
        v2b_t = [load(const, d) for d in v2b_d]
        bcol_t = load(const, bcol_d)
        ident_t = load(const, ident_d)
        w1ab_t = load(const, w1ab_d)
        w1cd_t = load(const, w1cd_d)
        wa_t = load(const, wa_d)
        wb_t = load(const, wb_d)
        wc_t = load(const, wc_d)
        wd_t = load(const, wd_d)

        # ---- MLP layer 1: h1 = relu(W0.T @ wT + b0) : [64, E] ----
        ps_h1 = ps_rot.tile([64, E], F32, tag="rot")
        nc.tensor.matmul(ps_h1[:], w0_t[:], wT_t[:], start=True, stop=True)
        h1_t = work.tile([64, E], BF16)
        nc.scalar.activation(
            h1_t[:], ps_h1[:], mybir.ActivationFunctionType.Relu,
            bias=b0c_t[:], scale=1.0,
        )

        # ---- persistent PSUM accumulators ----
        psA = ps_acc.tile([128, E], F32, tag="A")
        psB = ps_acc.tile([64, E], F32, tag="B")
        psC = [ps_acc.tile([64, E], F32, tag=f"C{i}", name=f"psC{i}") for i in range(3)]
        psD = ps_acc.tile([128, E], F32, tag="D")

        # bias chunks open each accumulation group
        nc.tensor.matmul(psA[:], ba_t[:], s1t_t[:], start=True, stop=False,
                         skip_group_check=True)
        nc.tensor.matmul(psB[:], bb_t[:], s1t_t[:], start=True, stop=False,
                         skip_group_check=True)
        for i in range(3):
            nc.tensor.matmul(psC[i][:], bc_t[:], vs_t[i][0:64, :], start=True,
                             stop=False, skip_group_check=True)
        nc.tensor.matmul(psD[:], bd_t[:], d2_t[0:64, :], start=True, stop=False,
                         skip_group_check=True)

        # ---- paths a/b: 64 chunks over joint (k, u), u full width ----
        for k in range(KAB):
            # broadcast h row k to 128 partitions: (W1[:,k] replicated).T @ h1
            ps_bc = ps_rot.tile([128, E], F32, tag="rot")
            nc.tensor.matmul(ps_bc[:], w1ab_t[:, bass.ts(k, 128)], h1_t[:],
                             start=True, stop=True, skip_group_check=True)
            bct = bct_pool.tile([128, E], BF16, tag="bct")
            nc.scalar.activation(
                bct[:], ps_bc[:], mybir.ActivationFunctionType.Relu,
                bias=b1ab_t[:, k : k + 1], scale=1.0,
            )
            pt = pt_pool.tile([128, E], BF16, tag="pt")
            nc.vector.tensor_mul(pt[:], s1t_t[:], bct[:])
            last = k == KAB - 1
            nc.tensor.matmul(psA[:], wa_t[:, bass.ts(k, 128)], pt[:],
                             start=False, stop=last, skip_group_check=True)
            nc.tensor.matmul(psB[:], wb_t[:, bass.ts(k, 64)], pt[:],
                             start=False, stop=last, skip_group_check=True)

        # ---- paths c/d: 32 chunks, each = 2 k-values x 64 u ----
        for g in range(GCD):
            ps_bc = ps_rot.tile([128, E], F32, tag="rot")
            nc.tensor.matmul(ps_bc[:], w1cd_t[:, bass.ts(g, 128)], h1_t[:],
                             start=True, stop=True, skip_group_check=True)
            bct = bct_pool.tile([128, E], BF16, tag="bct")
            nc.scalar.activation(
                bct[:], ps_bc[:], mybir.ActivationFunctionType.Relu,
                bias=b1cd_t[:, g : g + 1], scale=1.0,
            )
            last = g == GCD - 1
            for i in range(3):
                pt = pt_pool.tile([128, E], BF16, tag="pt")
                nc.vector.tensor_mul(pt[:], vs_t[i][:], bct[:])
                nc.tensor.matmul(psC[i][:], wc_t[:, bass.ts(g, 64)], pt[:],
                                 start=False, stop=last, skip_group_check=True)
            ptd = pt_pool.tile([128, E], BF16, tag="pt")
            nc.vector.tensor_mul(ptd[:], d2_t[:], bct[:])
            nc.tensor.matmul(psD[:], wd_t[:, bass.ts(g, 128)], ptd[:],
                             start=False, stop=last, skip_group_check=True)

        # ---- epilogue (still transposed): ----
        # out0T = (psA*s2) + I3*psD + bias ;  out1T_i = psB*v2_i + psC_i
        tA = work.tile([128, E], F32)
        nc.vector.tensor_mul(tA[:], psA[:], s2b_t[:])
        tD = work.tile([128, E], F32)
        nc.scalar.mul(tD[:], psD[:], I3)
        out0T = work.tile([128, E], F32)
        nc.vector.affine_then_add(out0T[:], tA[:], tD[:], scale=1.0,
                                  bias=bcol_t[:])
        out1T = []
        for i in range(3):
            tB = work.tile([64, E], F32, tag=f"tB{i}")
            nc.vector.tensor_mul(tB[:], psB[:], v2b_t[i][:])
            o1 = work.tile([64, E], F32, tag=f"o1{i}")
            nc.vector.affine_then_add(o1[:], tB[:], psC[i][:], scale=1.0,
                                      bias=0.0)
            out1T.append(o1)

        # ---- transpose back to [E, 320] and store ----
        for et in range(E // 128):
            sl = bass.ts(et, 128)
            outS = out_pool.tile([128, 320], F32, tag="outS")
            ps_t0 = ps_rot.tile([128, E], F32, tag="rot")
            nc.tensor.transpose(ps_t0[:, 0:128], out0T[:, sl], ident_t[:])
            nc.vector.tensor_copy(outS[:, 0:128], ps_t0[:, 0:128])
            o1v = outS[:, 128:320].rearrange("p (w i) -> p i w", i=3)
            for i in range(3):
                ps_ti = ps_rot.tile([128, E], F32, tag="rot")
                nc.tensor.transpose(ps_ti[:, 0:64], out1T[i][:, sl],
                                    ident_t[0:64, 0:64])
                nc.vector.tensor_copy(o1v[:, i, :], ps_ti[:, 0:64])
            nc.sync.dma_start(outp_d[sl, :], outS[:])

    nc.compile()
    return nc


_NC = None


def _get_nc():
    global _NC
    if _NC is None:
        _NC = _build_nc()
    return _NC


def _prep_inputs(data_in1, data_in2, weight, W0, b0, W1, b1, W2, b2, bias):
    f32 = np.float32
    data_in1 = np.ascontiguousarray(data_in1, dtype=f32)
    data_in2 = np.ascontiguousarray(data_in2, dtype=f32)
    weight = np.ascontiguousarray(weight, dtype=f32)
    W0 = np.asarray(W0, f32); b0 = np.asarray(b0, f32)
    W1 = np.asarray(W1, f32); b1 = np.asarray(b1, f32)
    W2 = np.asarray(W2, f32); b2 = np.asarray(b2, f32)
    bias = np.asarray(bias, f32)

    s1 = data_in1[:, :MUL0]                      # [N,128]
    v1 = data_in1[:, MUL0:].reshape(N, MUL1, 3)  # [N,64,3]
    s2 = data_in2[:, 0]                          # [N]
    v2 = data_in2[:, 1:4]                        # [N,3]

    def bf(x):
        return np.ascontiguousarray(x, dtype=f32).astype(BF16_NP)

    s1t = bf(s1.T)                               # [128,N]
    vs = []
    for i in range(3):
        v1s2 = (v1[:, :, i] * s2[:, None]).T     # [64,N]
        vs.append(bf(np.concatenate([v1s2, v1s2], axis=0)))
    dot12 = np.einsum("eui,ei->eu", v1, v2).T    # [64,N]
    d2 = bf(np.concatenate([dot12, dot12], axis=0))
    wT = bf(weight.T)                            # [16,N]
    s2b = np.ascontiguousarray(np.broadcast_to(s2, (128, N)), dtype=f32)
    v2b = [
        np.ascontiguousarray(np.broadcast_to(v2[:, i], (64, N)), dtype=f32)
        for i in range(3)
    ]

    shared = {
        "w0": bf(W0),
        "b0c": np.ascontiguousarray(b0.reshape(64, 1), f32),
        "w1ab": bf(np.repeat(W1, 128, axis=1)),
        "w1cd": bf(np.repeat(W1, 64, axis=1)),
        "b1ab": np.ascontiguousarray(np.broadcast_to(b1, (128, 64)), f32),
        "b1cd": np.concatenate(
            [
                np.broadcast_to(b1[0::2], (64, 32)),
                np.broadcast_to(b1[1::2], (64, 32)),
            ],
            axis=0,
        ).astype(f32),
        "wa": bf(W2[:, :N1].reshape(64, 128, 128).transpose(1, 0, 2)
                 .reshape(128, KAB * 128)),
        "wb": bf(W2[:, N1:N1 + N2].reshape(64, 128, 64).transpose(1, 0, 2)
                 .reshape(128, KAB * 64)),
        "wc": bf(W2[:, N1 + N2:N1 + N2 + N3].reshape(32, 2, 64, 64)
                 .transpose(1, 2, 0, 3).reshape(128, GCD * 64)),
        "wd": bf(W2[:, N1 + N2 + N3:].reshape(32, 2, 64, 128)
                 .transpose(1, 2, 0, 3).reshape(128, GCD * 128)),
        "ba": bf(b2[:N1].reshape(128, 128)),
        "bb": bf(b2[N1:N1 + N2].reshape(128, 64)),
        "bc": bf(b2[N1 + N2:N1 + N2 + N3].reshape(64, 64)),
        "bd": bf(b2[N1 + N2 + N3:].reshape(64, 128)),
        "bcol": np.ascontiguousarray(bias.reshape(128, 1), f32),
        "ident": np.eye(128, dtype=f32),
    }

    in_maps = []
    for c in range(N_CORES):
        e0 = c * E
        m = dict(shared)
        m["s1t"] = np.ascontiguousarray(s1t[:, e0:e0 + E])
        for i in range(3):
            m[f"vs{i}"] = np.ascontiguousarray(vs[i][:, e0:e0 + E])
            m[f"v2b{i}"] = np.ascontiguousarray(v2b[i][:, e0:e0 + E])
        m["d2"] = np.ascontiguousarray(d2[:, e0:e0 + E])
        m["wT"] = np.ascontiguousarray(wT[:, e0:e0 + E])
        m["s2b"] = np.ascontiguousarray(s2b[:, e0:e0 + E])
        in_maps.append(m)
    return in_maps


def run(in_maps, **kwargs):
    nc = _get_nc()
    return run_bass_kernel_spmd(nc, in_maps, list(range(N_CORES)), **kwargs)


def kernel(data_in1, data_in2, weight, W0, b0, W1, b1, W2, b2, bias):
    in_maps = _prep_inputs(
        data_in1, data_in2, weight, W0, b0, W1, b1, W2, b2, bias
    )
    res = run(in_maps)
    out = np.concatenate(
        [np.asarray(res.results[c]["outp"]) for c in range(N_CORES)], axis=0
    )
    return out.astype(np.float32)


# revision 6
# speedup vs baseline: 20742.7552x; 20742.7552x over previous
"""Trainium2 Bass kernel for nn_O3TensorProductWeighted.

Computes, for each sample e:
    h  = relu(relu(weight @ W0 + b0) @ W1 + b1)           # [64]
    w  = h @ W2 + b2                                      # [36864] (never materialized)
    out0 = PW0*(einsum(Wa,s1)*s2 + I3*einsum(Wd,dot12))
    out1 = PW1*I3*(einsum(Wb,s1) x v2 + einsum(Wc,v1)*s2)
    out  = concat(out0, out1)/SQRT_K ; out[:128] += bias

Strategy: reassociate each einsum against the (k,u)-joint contraction of the
per-sample Khatri-Rao product h (x) x, so everything becomes dense matmuls
over shared W2 chunk weights, with the per-sample products built on-chip.
All paths share 32 paired h-row broadcasts (PE replicated-W1-column matmuls
+ ACT relu evacuation), one fused DVE multiply per chunk produces the six
path operands at once, and four PSUM accumulation chains collect the
outputs. Runs in transposed layout (features on partitions, samples on the
free dim), 512 samples per core, pure data parallel over 8 cores.
"""

import dataclasses
import sys

sys.path.insert(0, "/opt/trn_rl_repo")

from contextlib import ExitStack

import ml_dtypes
import numpy as np

import concourse.bacc as bacc
import concourse.bass as bass
import concourse.tile as tile
from concourse import mybir
from concourse.bass_utils import run_bass_kernel_spmd

BF16 = mybir.dt.bfloat16
F32 = mybir.dt.float32
BF16_NP = ml_dtypes.bfloat16

N_CORES = 8
N = 4096
E = N // N_CORES  # 512 samples per core

MUL0, MUL1 = 128, 64
N1 = MUL0 * MUL0          # 16384
N2 = MUL0 * MUL1          # 8192
N3 = MUL1 * MUL1          # 4096
FAN = MUL0 + MUL1         # 192
I3 = float(1.0 / np.sqrt(3.0))
# PW0/SQRT_K == 1.0 and PW1*I3/SQRT_K == 1.0 exactly; only I3 remains on D.

G = 32  # chunks; chunk g covers k in {2g, 2g+1} x 64 u-values (128 rows)


def _build_nc():
    nc = bacc.Bacc(None)

    # per-core inputs, transposed [feature, E]
    s1t_d = nc.declare_dram_parameter("s1t", [128, E], BF16, isOutput=False)
    # fused TT operand: [s1lo2 | s1hi2 | vs0 | vs1 | vs2 | d2], each [128, E]
    fin0_d = nc.declare_dram_parameter("fin0", [128, 6 * E], BF16, isOutput=False)
    wT_d = nc.declare_dram_parameter("wT", [16, E], BF16, isOutput=False)
    s2b_d = nc.declare_dram_parameter("s2b", [128, E], F32, isOutput=False)
    v2b_d = [
        nc.declare_dram_parameter(f"v2b{i}", [64, E], F32, isOutput=False)
        for i in range(3)
    ]

    # replicated parameters
    w0_d = nc.declare_dram_parameter("w0", [16, 64], BF16, isOutput=False)
    b0c_d = nc.declare_dram_parameter("b0c", [64, 1], F32, isOutput=False)
    wg1_d = nc.declare_dram_parameter("wg1", [64, G * 128], BF16, isOutput=False)
    bg1_d = nc.declare_dram_parameter("bg1", [128, G], F32, isOutput=False)
    walo_d = nc.declare_dram_parameter("walo", [128, G * 128], BF16, isOutput=False)
    wahi_d = nc.declare_dram_parameter("wahi", [128, G * 128], BF16, isOutput=False)
    wblo_d = nc.declare_dram_parameter("wblo", [128, G * 64], BF16, isOutput=False)
    wbhi_d = nc.declare_dram_parameter("wbhi", [128, G * 64], BF16, isOutput=False)
    wc_d = nc.declare_dram_parameter("wc", [128, G * 64], BF16, isOutput=False)
    wd_d = nc.declare_dram_parameter("wd", [128, G * 128], BF16, isOutput=False)
    ba_d = nc.declare_dram_parameter("ba", [128, 128], BF16, isOutput=False)
    bb_d = nc.declare_dram_parameter("bb", [128, 64], BF16, isOutput=False)
    bc_d = nc.declare_dram_parameter("bc", [64, 64], BF16, isOutput=False)
    bd_d = nc.declare_dram_parameter("bd", [64, 128], BF16, isOutput=False)
    bcol_d = nc.declare_dram_parameter("bcol", [128, 1], F32, isOutput=False)
    ident_d = nc.declare_dram_parameter("ident", [128, 128], F32, isOutput=False)

    outp_d = nc.declare_dram_parameter("outp", [E, 320], F32, isOutput=True)

    with tile.TileContext(nc) as tc, ExitStack() as ctx:
        const = ctx.enter_context(tc.tile_pool(name="const", bufs=1))
        work = ctx.enter_context(tc.tile_pool(name="work", bufs=1))
        bct_pool = ctx.enter_context(tc.tile_pool(name="bct", bufs=4))
        pt_pool = ctx.enter_context(tc.tile_pool(name="pt", bufs=4))
        out_pool = ctx.enter_context(tc.tile_pool(name="outs", bufs=2))
        ps_acc = ctx.enter_context(tc.tile_pool(name="ps_acc", bufs=1, space="PSUM"))
        ps_rot = ctx.enter_context(tc.tile_pool(name="ps_rot", bufs=2, space="PSUM"))

        dma_engines = [nc.sync, nc.gpsimd]
        dma_i = [0]

        def load(dparam, engine=None):
            t = const.tile(dparam.shape, dparam.dtype, name=f"t_{dparam.name}")
            e = engine
            if e is None:
                e = dma_engines[dma_i[0] % len(dma_engines)]
                dma_i[0] += 1
            e.dma_start(t[:], dparam[:])
            return t

        # small inputs first so the MLP + chunk 0 can start immediately
        wT_t = load(wT_d)
        w0_t = load(w0_d, nc.sync)
        b0c_t = load(b0c_d, nc.sync)
        wg1_t = load(wg1_d, nc.sync)
        bg1_t = load(bg1_d, nc.sync)
        fin0_t = load(fin0_d)
        s1t_t = load(s1t_d)
        ba_t = load(ba_d)
        bb_t = load(bb_d)
        bc_t = load(bc_d)
        bd_t = load(bd_d)
        walo_t = load(walo_d, nc.gpsimd)
        wblo_t = load(wblo_d, nc.sync)
        wahi_t = load(wahi_d, nc.gpsimd)
        wbhi_t = load(wbhi_d, nc.sync)
        wc_t = load(wc_d, nc.sync)
        wd_t = load(wd_d, nc.gpsimd)
        s2b_t = load(s2b_d)
        v2b_t = [load(d) for d in v2b_d]
        bcol_t = load(bcol_d)
        ident_t = load(ident_d)

        # MLP layer 1: h1 = relu(W0.T @ wT + b0) : [64, E]
        ps_h1 = ps_rot.tile([64, E], F32, tag="rot")
        nc.tensor.matmul(ps_h1[:], w0_t[:], wT_t[:], start=True, stop=True)
        h1_t = work.tile([64, E], BF16)
        nc.scalar.activation(
            h1_t[:], ps_h1[:], mybir.ActivationFunctionType.Relu,
            bias=b0c_t[:], scale=1.0,
        )

        # persistent PSUM accumulators
        psA = ps_acc.tile([128, E], F32, tag="A")
        psB = ps_acc.tile([64, E], F32, tag="B")
        psC = [ps_acc.tile([64, E], F32, tag=f"C{i}", name=f"psC{i}")
               for i in range(3)]
        psD = ps_acc.tile([128, E], F32, tag="D")

        # bias chunks open each accumulation group
        f3 = fin0_t[:].rearrange("p (b e) -> p b e", b=6)
        nc.tensor.matmul(psA[:], ba_t[:], s1t_t[:], start=True, stop=False,
                         skip_group_check=True)
        nc.tensor.matmul(psB[:], bb_t[:], s1t_t[:], start=True, stop=False,
                         skip_group_check=True)
        for i in range(3):
            nc.tensor.matmul(psC[i][:], bc_t[:], f3[0:64, 2 + i, :], start=True,
                             stop=False, skip_group_check=True)
        nc.tensor.matmul(psD[:], bd_t[:], f3[0:64, 5, :], start=True, stop=False,
                         skip_group_check=True)

        # main loop: 32 chunks, each = 2 k-values; one broadcast serves all
        # six path operands.
        for g in range(G):
            ps_bc = ps_rot.tile([128, E], F32, tag="rot")
            nc.tensor.matmul(ps_bc[:], wg1_t[:, bass.ts(g, 128)], h1_t[:],
                             start=True, stop=True, skip_group_check=True)
            bct = bct_pool.tile([128, E], BF16, tag="bct")
            nc.scalar.activation(
                bct[:], ps_bc[:], mybir.ActivationFunctionType.Relu,
                bias=bg1_t[:, g : g + 1], scale=1.0,
            )
            # fused Khatri-Rao products: pt[:, j*E:(j+1)*E] = fin0_j * bct
            pt = pt_pool.tile([128, 6 * E], BF16, tag="pt")
            bct_b = dataclasses.replace(
                bct[:], ap=[bct[:].ap[0], [0, 6], [1, E]]
            )
            nc.vector.tensor_mul(
                pt[:].rearrange("p (b e) -> p b e", b=6), f3, bct_b
            )
            last = g == G - 1
            p3 = pt[:].rearrange("p (b e) -> p b e", b=6)
            nc.tensor.matmul(psA[:], walo_t[:, bass.ts(g, 128)], p3[:, 0, :],
                             start=False, stop=False, skip_group_check=True)
            nc.tensor.matmul(psA[:], wahi_t[:, bass.ts(g, 128)], p3[:, 1, :],
                             start=False, stop=last, skip_group_check=True)
            nc.tensor.matmul(psB[:], wblo_t[:, bass.ts(g, 64)], p3[:, 0, :],
                             start=False, stop=False, skip_group_check=True)
            nc.tensor.matmul(psB[:], wbhi_t[:, bass.ts(g, 64)], p3[:, 1, :],
                             start=False, stop=last, skip_group_check=True)
            for i in range(3):
                nc.tensor.matmul(psC[i][:], wc_t[:, bass.ts(g, 64)],
                                 p3[:, 2 + i, :], start=False, stop=last,
                                 skip_group_check=True)
            nc.tensor.matmul(psD[:], wd_t[:, bass.ts(g, 128)], p3[:, 5, :],
                             start=False, stop=last, skip_group_check=True)

        # epilogue (still transposed):
        # out0T = (psA*s2) + I3*psD + bias ;  out1T_i = psB*v2_i + psC_i
        tA = work.tile([128, E], F32)
        nc.vector.tensor_mul(tA[:], psA[:], s2b_t[:])
        tD = work.tile([128, E], F32)
        nc.scalar.mul(tD[:], psD[:], I3)
        out0T = work.tile([128, E], F32)
        nc.vector.affine_then_add(out0T[:], tA[:], tD[:], scale=1.0,
                                  bias=bcol_t[:])
        out1T = []
        for i in range(3):
            tB = work.tile([64, E], F32, tag=f"tB{i}")
            nc.vector.tensor_mul(tB[:], psB[:], v2b_t[i][:])
            o1 = work.tile([64, E], F32, tag=f"o1{i}")
            nc.vector.affine_then_add(o1[:], tB[:], psC[i][:], scale=1.0,
                                      bias=0.0)
            out1T.append(o1)

        # transpose back to [E, 320] and store
        for et in range(E // 128):
            sl = bass.ts(et, 128)
            outS = out_pool.tile([128, 320], F32, tag="outS")
            ps_t0 = ps_rot.tile([128, E], F32, tag="rot")
            nc.tensor.transpose(ps_t0[:, 0:128], out0T[:, sl], ident_t[:])
            nc.scalar.copy(outS[:, 0:128], ps_t0[:, 0:128])
            o1v = outS[:, 128:320].rearrange("p (w i) -> p i w", i=3)
            for i in range(3):
                ps_ti = ps_rot.tile([128, E], F32, tag="rot")
                nc.tensor.transpose(ps_ti[:, 0:64], out1T[i][:, sl],
                                    ident_t[0:64, 0:64])
                nc.scalar.copy(o1v[:, i, :], ps_ti[:, 0:64])
            nc.sync.dma_start(outp_d[sl, :], outS[:])

    nc.compile()
    return nc


_NC = None


def _get_nc():
    global _NC
    if _NC is None:
        _NC = _build_nc()
    return _NC


def _prep_inputs(data_in1, data_in2, weight, W0, b0, W1, b1, W2, b2, bias):
    f32 = np.float32
    data_in1 = np.ascontiguousarray(data_in1, dtype=f32)
    data_in2 = np.ascontiguousarray(data_in2, dtype=f32)
    weight = np.ascontiguousarray(weight, dtype=f32)
    W0 = np.asarray(W0, f32); b0 = np.asarray(b0, f32)
    W1 = np.asarray(W1, f32); b1 = np.asarray(b1, f32)
    W2 = np.asarray(W2, f32); b2 = np.asarray(b2, f32)
    bias = np.asarray(bias, f32)

    s1 = data_in1[:, :MUL0]                      # [N,128]
    v1 = data_in1[:, MUL0:].reshape(N, MUL1, 3)  # [N,64,3]
    s2 = data_in2[:, 0]                          # [N]
    v2 = data_in2[:, 1:4]                        # [N,3]

    def bf(x):
        return np.ascontiguousarray(x, dtype=f32).astype(BF16_NP)

    s1t = s1.T                                   # [128,N] f32
    # fused TT operand blocks, each [128, N]
    s1lo = np.concatenate([s1t[0:64], s1t[0:64]], axis=0)
    s1hi = np.concatenate([s1t[64:128], s1t[64:128]], axis=0)
    vs = []
    for i in range(3):
        v1s2 = (v1[:, :, i] * s2[:, None]).T     # [64,N]
        vs.append(np.concatenate([v1s2, v1s2], axis=0))
    dot12 = np.einsum("eui,ei->eu", v1, v2).T    # [64,N]
    d2 = np.concatenate([dot12, dot12], axis=0)
    fin0 = bf(np.stack([s1lo, s1hi, vs[0], vs[1], vs[2], d2], axis=1))
    # fin0: [128, 6, N]
    wT = bf(weight.T)
    s2b = np.ascontiguousarray(np.broadcast_to(s2, (128, N)), dtype=f32)
    v2b = [
        np.ascontiguousarray(np.broadcast_to(v2[:, i], (64, N)), dtype=f32)
        for i in range(3)
    ]

    # W2 chunk layouts: chunk g rows r=(koff*64+uu) = W2x[2g+koff, sel(uu), :]
    def chunks(arr3, usel):  # arr3 [64,U,W] -> [128, G*W]
        a = arr3.reshape(G, 2, arr3.shape[1], arr3.shape[2])[:, :, usel, :]
        return bf(np.transpose(a, (1, 2, 0, 3)).reshape(128, -1))

    Wa3 = W2[:, :N1].reshape(64, 128, 128)
    Wb3 = W2[:, N1:N1 + N2].reshape(64, 128, 64)
    Wc3 = W2[:, N1 + N2:N1 + N2 + N3].reshape(64, 64, 64)
    Wd3 = W2[:, N1 + N2 + N3:].reshape(64, 64, 128)
    lo, hi = slice(0, 64), slice(64, 128)

    shared = {
        "w0": bf(W0),
        "b0c": np.ascontiguousarray(b0.reshape(64, 1), f32),
        "wg1": bf(np.repeat(W1, 64, axis=1)),
        "bg1": np.concatenate(
            [np.broadcast_to(b1[0::2], (64, G)),
             np.broadcast_to(b1[1::2], (64, G))], axis=0).astype(f32),
        "walo": chunks(Wa3, lo),
        "wahi": chunks(Wa3, hi),
        "wblo": chunks(Wb3, lo),
        "wbhi": chunks(Wb3, hi),
        "wc": chunks(Wc3, lo),
        "wd": chunks(Wd3, lo),
        "ba": bf(b2[:N1].reshape(128, 128)),
        "bb": bf(b2[N1:N1 + N2].reshape(128, 64)),
        "bc": bf(b2[N1 + N2:N1 + N2 + N3].reshape(64, 64)),
        "bd": bf(b2[N1 + N2 + N3:].reshape(64, 128)),
        "bcol": np.ascontiguousarray(bias.reshape(128, 1), f32),
        "ident": np.eye(128, dtype=f32),
    }

    in_maps = []
    for c in range(N_CORES):
        e0 = c * E
        m = dict(shared)
        m["s1t"] = bf(s1t[:, e0:e0 + E])
        m["fin0"] = np.ascontiguousarray(
            fin0[:, :, e0:e0 + E]).reshape(128, 6 * E)
        m["wT"] = np.ascontiguousarray(wT[:, e0:e0 + E])
        m["s2b"] = np.ascontiguousarray(s2b[:, e0:e0 + E])
        for i in range(3):
            m[f"v2b{i}"] = np.ascontiguousarray(v2b[i][:, e0:e0 + E])
        in_maps.append(m)
    return in_maps


def run(in_maps, **kwargs):
    nc = _get_nc()
    return run_bass_kernel_spmd(nc, in_maps, list(range(N_CORES)), **kwargs)


def kernel(data_in1, data_in2, weight, W0, b0, W1, b1, W2, b2, bias):
    in_maps = _prep_inputs(
        data_in1, data_in2, weight, W0, b0, W1, b1, W2, b2, bias
    )
    res = run(in_maps)
    out = np.concatenate(
        [np.asarray(res.results[c]["outp"]) for c in range(N_CORES)], axis=0
    )
    return out.astype(np.float32)
